# revision 14
# baseline (speedup 1.0000x reference)
"""Trainium2 Bass kernel for nn_ContrastiveLoss (N=4096, D=128, NT=512, Q=8).

Strategy (8 NeuronCores, data parallel over N, no cross-core collective):
  - Each core owns R = N/8 = 512 rows of x (4 chunks of 128 partitions).
  - Host pre-transposes x and yf (=y.reshape(N,D)) to bf16 so both matmul
    operands load contiguously as [D=128 partitions, N] tiles.
  - Per (half h, chunk cc) the core computes S_xx = x_chunk @ x[:,half].T and
    S_xy = x_chunk @ yf[:,half].T on the PE in bf16 (K=D=128), exp(S/T) on
    the ACT engine. The xy exp uses the fused per-partition accumulator for
    its row sums; the xx tile goes through one fused DVE
    scalar_tensor_tensor ((tcol != trow) * E with accumulator) for the
    same-track-masked row sums.
  - Device output is just the 16 raw accumulator columns per 128-row chunk:
    [toty_h0, toty_h1, denx_h0, denx_h1] x 4 chunks -> [128, 16] f32.
  - Everything O(N) or smaller lives on the host combine step (this is the
    "all-reduce num/den across devices" of the sharding hint): den_j =
    toty_j + denx_j - own_j, the positive-pair path (sim_p/num/own from a
    [N,Q,D] einsum), and the pair term SUM_ij log(den_j + num_i) via the
    log1p series
      N*SUM_j log den_j + SUM_k (-1)^(k+1)/k (SUM_i num_i^k)(SUM_j den_j^-k)
    in f64, with an exact-evaluation fallback if the series tail is not
    vanishing (cannot happen for unit-norm inputs).
  - DMA is issued from four engine queues in parallel (SP/Pool/DVE/ACT all
    have DGE slots) in need-order, h-major, so the first matmul only waits
    on xsh + xT[:, :1024] and the backbone never starves.
"""

import numpy as np
import ml_dtypes

import concourse.bass as bass
import concourse.bacc as bacc
import concourse.tile as tile
import concourse.mybir as mybir
from concourse import bass_utils

P = 128           # partitions / rows per chunk
N = 4096          # total rows of x
D = 128           # feature dim
NT = 512          # number of tracks
Q = 8             # views per track
CORES = 8
R = N // CORES    # rows per core = 512
NCH = R // P      # chunks per core = 4
TEMP = 0.05
INV_T = 1.0 / TEMP
HALF = 2048       # exp tile width (4 PSUM banks)
NH = N // HALF    # 2 halves
K_SER = 6         # log1p series order (host side, f64)
ACC_C = 4 * NCH   # 16 output columns
F32 = mybir.dt.float32
BF16 = mybir.dt.bfloat16
FP8 = mybir.dt.float8e4
ALU = mybir.AluOpType
ACTF = mybir.ActivationFunctionType
AX = mybir.AxisListType

_CACHE = {}


def _build():
    nc = bacc.Bacc("TRN2", target_bir_lowering=False, debug=False,
                   num_devices=CORES)

    xT_d = nc.dram_tensor("xT", [D, N], FP8, kind="ExternalInput")
    yT_d = nc.dram_tensor("yT", [D, N], FP8, kind="ExternalInput")
    xsh_d = nc.dram_tensor("xsh", [D, R], FP8, kind="ExternalInput")
    # trow: per-chunk track ids, [p, cc] = track[512*c + 128*cc + p] - 256
    trow_d = nc.dram_tensor("trow", [P, NCH], F32, kind="ExternalInput")
    # track id - 256 of every x column (broadcast across partitions on device)
    tcol_d = nc.dram_tensor("tcol", [1, N], BF16, kind="ExternalInput")
    out_d = nc.dram_tensor("out", [P, ACC_C], F32, kind="ExternalOutput")

    with tile.TileContext(nc) as tc:
        with (
            tc.tile_pool(name="persist", bufs=1) as pp,
            tc.tile_pool(name="escr", bufs=3) as ep,
            tc.tile_pool(name="sttjunk", bufs=2) as sjp,
            tc.tile_pool(name="psum", bufs=2, space="PSUM") as psp,
        ):
            # ---- persistent tiles ----
            xT_s = pp.tile([D, N], FP8, tag="xT_s")
            yT_s = pp.tile([D, N], FP8, tag="yT_s")
            xsh_s = pp.tile([D, R], FP8, tag="xsh_s")
            trow_s = pp.tile([P, NCH], F32, tag="trow_s")
            tcol_s = pp.tile([P, N], BF16, tag="tcol_s")
            acc_s = pp.tile([P, ACC_C], F32, tag="acc_s")

            # ---- input loads: two DGE rings (SP/sync, Pool/gpsimd), in
            # need-order. h-major backbone: first half needs only cols
            # [0:2048] of (rolled) xT, then yT; second half the rest. xT is
            # rolled per-core so its first 512 columns equal xsh, letting
            # the first matmul start from xsh alone. Keep the ACT queue
            # free of DGE work. ----
            Hq = N // 4  # 1024-col DMA slices
            HH = Hq // 2  # 512-col slices for the first epoch
            # epoch 1: xsh + xx0 columns, balanced across both rings
            nc.sync.dma_start(out=xsh_s[:], in_=xsh_d.ap())
            nc.gpsimd.dma_start(out=xT_s[:, R:2 * HH],
                                in_=xT_d.ap()[:, R:2 * HH])
            nc.sync.dma_start(out=xT_s[:, 2 * HH:3 * HH],
                              in_=xT_d.ap()[:, 2 * HH:3 * HH])
            nc.gpsimd.dma_start(out=xT_s[:, 3 * HH:4 * HH],
                                in_=xT_d.ap()[:, 3 * HH:4 * HH])
            # epoch 2: xy0 columns, on the otherwise-idle ACT DGE ring so
            # they don't contend with the xx columns above
            nc.scalar.dma_start(out=yT_s[:, 0:Hq], in_=yT_d.ap()[:, 0:Hq])
            nc.scalar.dma_start(out=yT_s[:, Hq:2 * Hq],
                                in_=yT_d.ap()[:, Hq:2 * Hq])
            # epoch 3: mask data for the h0 STTs (DVE has slack)
            nc.sync.dma_start(out=trow_s[:], in_=trow_d.ap())
            nc.gpsimd.dma_start(
                out=tcol_s[:, 0:HALF],
                in_=tcol_d.ap()[0:1, 0:HALF].to_broadcast([P, HALF]))
            # epoch 4: second-half columns (needed ~15 tile-periods in)
            nc.sync.dma_start(out=xT_s[:, 2 * Hq:3 * Hq],
                              in_=xT_d.ap()[:, 2 * Hq:3 * Hq])
            nc.gpsimd.dma_start(out=xT_s[:, 3 * Hq:4 * Hq],
                                in_=xT_d.ap()[:, 3 * Hq:4 * Hq])
            nc.sync.dma_start(out=yT_s[:, 2 * Hq:3 * Hq],
                              in_=yT_d.ap()[:, 2 * Hq:3 * Hq])
            nc.gpsimd.dma_start(out=yT_s[:, 3 * Hq:4 * Hq],
                                in_=yT_d.ap()[:, 3 * Hq:4 * Hq])
            nc.gpsimd.dma_start(
                out=tcol_s[:, HALF:N],
                in_=tcol_d.ap()[0:1, HALF:N].to_broadcast([P, HALF]))

            # ---- backbone: h-major so the first 8 tiles reuse cols [0:2048]
            for h in range(NH):
                for cc in range(NCH):
                    lhsT = xsh_s[:, cc * P:(cc + 1) * P]
                    # --- xx (rolled cols; [0:512) lives in xsh_s) ---
                    ps = psp.tile([P, HALF], F32, tag="ps")
                    for k in range(HALF // 512):
                        lo = HALF * h + 512 * k
                        rhs = (xsh_s[:, 0:512] if lo == 0
                               else xT_s[:, lo:lo + 512])
                        nc.tensor.matmul(out=ps[:, 512 * k:512 * (k + 1)],
                                         lhsT=lhsT, rhs=rhs,
                                         start=True, stop=True)
                    e = ep.tile([P, HALF], BF16, tag="escr")
                    nc.scalar.activation(out=e[:], in_=ps[:],
                                         func=ACTF.Exp, scale=INV_T)
                    sj = sjp.tile([P, HALF], BF16, tag="sttjunk")
                    nc.vector.scalar_tensor_tensor(
                        out=sj[:],
                        in0=tcol_s[:, HALF * h:HALF * (h + 1)],
                        scalar=trow_s[:, cc:cc + 1],
                        in1=e[:],
                        op0=ALU.not_equal,
                        op1=ALU.mult,
                        accum_out=acc_s[:, 4 * cc + 2 + h:4 * cc + 3 + h],
                    )
                    # --- xy ---
                    ps2 = psp.tile([P, HALF], F32, tag="ps")
                    for k in range(HALF // 512):
                        sl = slice(HALF * h + 512 * k, HALF * h + 512 * (k + 1))
                        nc.tensor.matmul(out=ps2[:, 512 * k:512 * (k + 1)],
                                         lhsT=lhsT, rhs=yT_s[:, sl],
                                         start=True, stop=True)
                    e2 = ep.tile([P, HALF], BF16, tag="escr")
                    nc.scalar.activation(
                        out=e2[:], in_=ps2[:], func=ACTF.Exp, scale=INV_T,
                        accum_out=acc_s[:, 4 * cc + h:4 * cc + 1 + h])

            # split the output DMA: chunks 0-2 finish ~4 tile-periods before
            # chunk 3, so only chunk 3's 4 columns ride the critical tail
            nc.sync.dma_start(out=out_d.ap()[:, 0:12], in_=acc_s[:, 0:12])
            nc.sync.dma_start(out=out_d.ap()[:, 12:16], in_=acc_s[:, 12:16])

    nc.compile()
    return nc


def get_nc():
    if "nc" not in _CACHE:
        _CACHE["nc"] = _build()
    return _CACHE["nc"]


def prepare_in_maps(x, track_idxs, y):
    x = np.ascontiguousarray(np.asarray(x), dtype=np.float32)
    y = np.ascontiguousarray(np.asarray(y), dtype=np.float32)
    t = np.asarray(track_idxs).astype(np.int64)
    xT = np.ascontiguousarray(x.T.astype(ml_dtypes.float8_e4m3fn))
    yT = np.ascontiguousarray(y.reshape(N, D).T.astype(ml_dtypes.float8_e4m3fn))
    tf = t.astype(np.float32)
    tcol = (tf - 256.0).astype(ml_dtypes.bfloat16)
    in_maps = []
    for c in range(CORES):
        rows = slice(c * R, (c + 1) * R)
        xsh = np.ascontiguousarray(xT[:, rows])
        trow = np.ascontiguousarray((tf[rows] - 256.0).reshape(NCH, P).T)
        # roll the xx columns so [0:512) == this core's own rows (= xsh)
        xTr = np.ascontiguousarray(np.roll(xT, -R * c, axis=1))
        tcolr = np.ascontiguousarray(np.roll(tcol, -R * c).reshape(1, N))
        in_maps.append({
            "xT": xTr, "yT": yT, "xsh": xsh, "trow": trow, "tcol": tcolr,
        })
    return in_maps


def _host_positive_path(x, track_idxs, y):
    """sim_p, num, own for every row, in f64 (O(N*Q*D) -- host-side prep)."""
    x = np.asarray(x, dtype=np.float64)
    y = np.asarray(y, dtype=np.float64)
    t = np.asarray(track_idxs).astype(np.int64)
    dots = np.einsum('nd,nqd->nq', x, y[t])          # [N, Q]
    sim_p = dots.min(axis=1)                         # [N]
    num = np.exp(sim_p / TEMP)
    own = np.exp(dots / TEMP).sum(axis=1)            # [N]
    return sim_p, num, own


def _exact_fallback(den, num, sim_p):
    loss = (np.log(den[None, :] + num[:, None]).mean()
            - (sim_p / TEMP).mean())
    return np.float32(loss)


def combine_outputs(outs, inputs):
    """outs: per-core [P, 16] accumulator arrays. Assemble den per row,
    combine with the host positive path via the log1p series (f64)."""
    sim_p, num, own = _host_positive_path(**inputs)
    den = np.empty(N, dtype=np.float64)
    for c, o in enumerate(outs):
        o = np.asarray(o, dtype=np.float64).reshape(P, NCH, 4)
        # per (p, cc): toty_h0 + toty_h1 + denx_h0 + denx_h1
        dsum = o.sum(axis=2)                         # [P, NCH]
        for cc in range(NCH):
            lo = c * R + cc * P
            den[lo:lo + P] = dsum[:, cc]
    den -= own
    if not (np.all(np.isfinite(den)) and np.all(den > 0)):
        return _exact_fallback(den, num, sim_p)
    logden = np.log(den).sum()
    terms = [(-1.0) ** (k + 1) / k * (num ** k).sum() * (den ** -k).sum()
             for k in range(1, K_SER + 1)]
    pair = N * logden + sum(terms)
    if not (abs(terms[-1]) <= 1e-8 * abs(pair) + 1e-12
            and abs(terms[-1]) <= abs(terms[-2]) + 1e-30):
        return _exact_fallback(den, num, sim_p)
    return np.float32(pair / (N * N) - sim_p.mean() / TEMP)


def kernel(x, track_idxs, y):
    nc = get_nc()
    in_maps = prepare_in_maps(x, track_idxs, y)
    res = bass_utils.run_bass_kernel_spmd(nc, in_maps,
                                          core_ids=list(range(CORES)))
    return combine_outputs([r["out"] for r in res.results],
                           inputs={"x": x, "track_idxs": track_idxs, "y": y})


if __name__ == "__main__":
    nc = get_nc()
    print("build + compile OK")


# revision 15
# speedup vs baseline: 1.0493x; 1.0493x over previous
"""Trainium2 Bass kernel for nn_ContrastiveLoss (N=4096, D=128, NT=512, Q=8).

Strategy (8 NeuronCores, data parallel over N, no cross-core collective):
  - Each core owns R = N/8 = 512 rows of x (4 chunks of 128 partitions).
  - Host pre-transposes x and yf (=y.reshape(N,D)) to bf16 so both matmul
    operands load contiguously as [D=128 partitions, N] tiles.
  - Per (half h, chunk cc) the core computes S_xx = x_chunk @ x[:,half].T and
    S_xy = x_chunk @ yf[:,half].T on the PE in bf16 (K=D=128), exp(S/T) on
    the ACT engine. The xy exp uses the fused per-partition accumulator for
    its row sums; the xx tile goes through one fused DVE
    scalar_tensor_tensor ((tcol != trow) * E with accumulator) for the
    same-track-masked row sums.
  - Device output is just the 16 raw accumulator columns per 128-row chunk:
    [toty_h0, toty_h1, denx_h0, denx_h1] x 4 chunks -> [128, 16] f32.
  - Everything O(N) or smaller lives on the host combine step (this is the
    "all-reduce num/den across devices" of the sharding hint): den_j =
    toty_j + denx_j - own_j, the positive-pair path (sim_p/num/own from a
    [N,Q,D] einsum), and the pair term SUM_ij log(den_j + num_i) via the
    log1p series
      N*SUM_j log den_j + SUM_k (-1)^(k+1)/k (SUM_i num_i^k)(SUM_j den_j^-k)
    in f64, with an exact-evaluation fallback if the series tail is not
    vanishing (cannot happen for unit-norm inputs).
  - DMA is issued from four engine queues in parallel (SP/Pool/DVE/ACT all
    have DGE slots) in need-order, h-major, so the first matmul only waits
    on xsh + xT[:, :1024] and the backbone never starves.
"""

import numpy as np
import ml_dtypes

import concourse.bass as bass
import concourse.bacc as bacc
import concourse.tile as tile
import concourse.mybir as mybir
from concourse import bass_utils

P = 128           # partitions / rows per chunk
N = 4096          # total rows of x
D = 128           # feature dim
NT = 512          # number of tracks
Q = 8             # views per track
CORES = 8
R = N // CORES    # rows per core = 512
NCH = R // P      # chunks per core = 4
TEMP = 0.05
INV_T = 1.0 / TEMP
HALF = 2048       # exp tile width (4 PSUM banks)
NH = N // HALF    # 2 halves
K_SER = 6         # log1p series order (host side, f64)
ACC_C = 4 * NCH   # 16 output columns
F32 = mybir.dt.float32
BF16 = mybir.dt.bfloat16
FP8 = mybir.dt.float8e4
ALU = mybir.AluOpType
ACTF = mybir.ActivationFunctionType
AX = mybir.AxisListType

_CACHE = {}


def _build():
    nc = bacc.Bacc("TRN2", target_bir_lowering=False, debug=False,
                   num_devices=CORES)

    xT_d = nc.dram_tensor("xT", [D, N], FP8, kind="ExternalInput")
    yT_d = nc.dram_tensor("yT", [D, N], FP8, kind="ExternalInput")
    xsh_d = nc.dram_tensor("xsh", [D, R], FP8, kind="ExternalInput")
    # trow: per-chunk track ids, [p, cc] = track[512*c + 128*cc + p] - 256
    trow_d = nc.dram_tensor("trow", [P, NCH], F32, kind="ExternalInput")
    # track id - 256 of every x column (broadcast across partitions on device)
    tcol_d = nc.dram_tensor("tcol", [1, N], BF16, kind="ExternalInput")
    out_d = nc.dram_tensor("out", [P, ACC_C], F32, kind="ExternalOutput")

    with tile.TileContext(nc) as tc:
        with (
            tc.tile_pool(name="persist", bufs=1) as pp,
            tc.tile_pool(name="escr", bufs=3) as ep,
            tc.tile_pool(name="sttjunk", bufs=2) as sjp,
            tc.tile_pool(name="psum", bufs=2, space="PSUM") as psp,
        ):
            # ---- persistent tiles ----
            xT_s = pp.tile([D, N], FP8, tag="xT_s")
            yT_s = pp.tile([D, N], FP8, tag="yT_s")
            xsh_s = pp.tile([D, R], FP8, tag="xsh_s")
            trow_s = pp.tile([P, NCH], F32, tag="trow_s")
            tcol_s = pp.tile([P, N], BF16, tag="tcol_s")
            acc_s = pp.tile([P, ACC_C], F32, tag="acc_s")

            # ---- input loads: two DGE rings (SP/sync, Pool/gpsimd), in
            # need-order. h-major backbone: first half needs only cols
            # [0:2048] of (rolled) xT, then yT; second half the rest. xT is
            # rolled per-core so its first 512 columns equal xsh, letting
            # the first matmul start from xsh alone. Keep the ACT queue
            # free of DGE work. ----
            # epoch 1: xsh alone on sync (first matmul), xx0 columns as one
            # transfer+semaphore on gpsimd, xy0 columns on the ACT ring
            nc.sync.dma_start(out=xsh_s[:], in_=xsh_d.ap())
            nc.gpsimd.dma_start(out=xT_s[:, R:HALF],
                                in_=xT_d.ap()[:, R:HALF])
            nc.scalar.dma_start(out=yT_s[:, 0:HALF], in_=yT_d.ap()[:, 0:HALF])
            # epoch 2: mask data for the h0 STTs (DVE has slack)
            nc.sync.dma_start(out=trow_s[:], in_=trow_d.ap())
            nc.gpsimd.dma_start(
                out=tcol_s[:, 0:HALF],
                in_=tcol_d.ap()[0:1, 0:HALF].to_broadcast([P, HALF]))
            # epoch 3: second-half columns (needed ~8 tile-periods in)
            nc.sync.dma_start(out=yT_s[:, HALF:N], in_=yT_d.ap()[:, HALF:N])
            nc.gpsimd.dma_start(out=xT_s[:, HALF:N],
                                in_=xT_d.ap()[:, HALF:N])
            nc.gpsimd.dma_start(
                out=tcol_s[:, HALF:N],
                in_=tcol_d.ap()[0:1, HALF:N].to_broadcast([P, HALF]))

            # ---- backbone: h-major so the first 8 tiles reuse cols [0:2048]
            for h in range(NH):
                for cc in range(NCH):
                    lhsT = xsh_s[:, cc * P:(cc + 1) * P]
                    # --- xx (rolled cols; [0:512) lives in xsh_s) ---
                    ps = psp.tile([P, HALF], F32, tag="ps")
                    for k in range(HALF // 512):
                        lo = HALF * h + 512 * k
                        rhs = (xsh_s[:, 0:512] if lo == 0
                               else xT_s[:, lo:lo + 512])
                        nc.tensor.matmul(out=ps[:, 512 * k:512 * (k + 1)],
                                         lhsT=lhsT, rhs=rhs,
                                         start=True, stop=True)
                    e = ep.tile([P, HALF], BF16, tag="escr")
                    nc.scalar.activation(out=e[:], in_=ps[:],
                                         func=ACTF.Exp, scale=INV_T)
                    sj = sjp.tile([P, HALF], BF16, tag="sttjunk")
                    nc.vector.scalar_tensor_tensor(
                        out=sj[:],
                        in0=tcol_s[:, HALF * h:HALF * (h + 1)],
                        scalar=trow_s[:, cc:cc + 1],
                        in1=e[:],
                        op0=ALU.not_equal,
                        op1=ALU.mult,
                        accum_out=acc_s[:, 4 * cc + 2 + h:4 * cc + 3 + h],
                    )
                    # --- xy ---
                    ps2 = psp.tile([P, HALF], F32, tag="ps")
                    for k in range(HALF // 512):
                        sl = slice(HALF * h + 512 * k, HALF * h + 512 * (k + 1))
                        nc.tensor.matmul(out=ps2[:, 512 * k:512 * (k + 1)],
                                         lhsT=lhsT, rhs=yT_s[:, sl],
                                         start=True, stop=True)
                    e2 = ep.tile([P, HALF], BF16, tag="escr")
                    nc.scalar.activation(
                        out=e2[:], in_=ps2[:], func=ACTF.Exp, scale=INV_T,
                        accum_out=acc_s[:, 4 * cc + h:4 * cc + 1 + h])

            # split the output DMA: chunks 0-2 finish ~4 tile-periods before
            # chunk 3, so only chunk 3's 4 columns ride the critical tail
            nc.sync.dma_start(out=out_d.ap()[:, 0:12], in_=acc_s[:, 0:12])
            nc.sync.dma_start(out=out_d.ap()[:, 12:16], in_=acc_s[:, 12:16])

    nc.compile()
    return nc


def get_nc():
    if "nc" not in _CACHE:
        _CACHE["nc"] = _build()
    return _CACHE["nc"]


def prepare_in_maps(x, track_idxs, y):
    x = np.ascontiguousarray(np.asarray(x), dtype=np.float32)
    y = np.ascontiguousarray(np.asarray(y), dtype=np.float32)
    t = np.asarray(track_idxs).astype(np.int64)
    xT = np.ascontiguousarray(x.T.astype(ml_dtypes.float8_e4m3fn))
    yT = np.ascontiguousarray(y.reshape(N, D).T.astype(ml_dtypes.float8_e4m3fn))
    tf = t.astype(np.float32)
    tcol = (tf - 256.0).astype(ml_dtypes.bfloat16)
    in_maps = []
    for c in range(CORES):
        rows = slice(c * R, (c + 1) * R)
        xsh = np.ascontiguousarray(xT[:, rows])
        trow = np.ascontiguousarray((tf[rows] - 256.0).reshape(NCH, P).T)
        # roll the xx columns so [0:512) == this core's own rows (= xsh)
        xTr = np.ascontiguousarray(np.roll(xT, -R * c, axis=1))
        tcolr = np.ascontiguousarray(np.roll(tcol, -R * c).reshape(1, N))
        in_maps.append({
            "xT": xTr, "yT": yT, "xsh": xsh, "trow": trow, "tcol": tcolr,
        })
    return in_maps


def _host_positive_path(x, track_idxs, y):
    """sim_p, num, own for every row, in f64 (O(N*Q*D) -- host-side prep)."""
    x = np.asarray(x, dtype=np.float64)
    y = np.asarray(y, dtype=np.float64)
    t = np.asarray(track_idxs).astype(np.int64)
    dots = np.einsum('nd,nqd->nq', x, y[t])          # [N, Q]
    sim_p = dots.min(axis=1)                         # [N]
    num = np.exp(sim_p / TEMP)
    own = np.exp(dots / TEMP).sum(axis=1)            # [N]
    return sim_p, num, own


def _exact_fallback(den, num, sim_p):
    loss = (np.log(den[None, :] + num[:, None]).mean()
            - (sim_p / TEMP).mean())
    return np.float32(loss)


def combine_outputs(outs, inputs):
    """outs: per-core [P, 16] accumulator arrays. Assemble den per row,
    combine with the host positive path via the log1p series (f64)."""
    sim_p, num, own = _host_positive_path(**inputs)
    den = np.empty(N, dtype=np.float64)
    for c, o in enumerate(outs):
        o = np.asarray(o, dtype=np.float64).reshape(P, NCH, 4)
        # per (p, cc): toty_h0 + toty_h1 + denx_h0 + denx_h1
        dsum = o.sum(axis=2)                         # [P, NCH]
        for cc in range(NCH):
            lo = c * R + cc * P
            den[lo:lo + P] = dsum[:, cc]
    den -= own
    if not (np.all(np.isfinite(den)) and np.all(den > 0)):
        return _exact_fallback(den, num, sim_p)
    logden = np.log(den).sum()
    terms = [(-1.0) ** (k + 1) / k * (num ** k).sum() * (den ** -k).sum()
             for k in range(1, K_SER + 1)]
    pair = N * logden + sum(terms)
    if not (abs(terms[-1]) <= 1e-8 * abs(pair) + 1e-12
            and abs(terms[-1]) <= abs(terms[-2]) + 1e-30):
        return _exact_fallback(den, num, sim_p)
    return np.float32(pair / (N * N) - sim_p.mean() / TEMP)


def kernel(x, track_idxs, y):
    nc = get_nc()
    in_maps = prepare_in_maps(x, track_idxs, y)
    res = bass_utils.run_bass_kernel_spmd(nc, in_maps,
                                          core_ids=list(range(CORES)))
    return combine_outputs([r["out"] for r in res.results],
                           inputs={"x": x, "track_idxs": track_idxs, "y": y})


if __name__ == "__main__":
    nc = get_nc()
    print("build + compile OK")
